# revision 19
# baseline (speedup 1.0000x reference)
"""DIMPA 2-hop directed message passing on 8 Trainium2 NeuronCores (Bass).

v2: deg-normalization folded into per-edge weights (w_norm = w * deg_inv[src]),
fp16 gather tables + fp16 one-hot matmuls, chunked dma_gather calls,
self-loops as a diagonal matmul (no gather slots).

Launches:
  A: per-core deg_inv (from weights grouped by node) + fp16 cast of own x rows.
  B: hop1 convs (gather x16, one-hot matmul accumulate, write c1 fp16).
  C: hop2 convs (gather c1 fp16) + feat epilogue -> out [N, 2D].
Host between launches only permutes / concatenates device-produced arrays.
"""

import os
import numpy as np
from concourse import bacc, mybir
import concourse.tile as tile
from concourse.bass_utils import run_bass_kernel_spmd

FILL = 0.5
NCORES = 8
P = 128
CH = 10                     # blocks per gather chunk
F32 = mybir.dt.float32
F16 = mybir.dt.float16
I16 = mybir.dt.int16
I32 = mybir.dt.int32

LAST_EXEC_NS = []
TRACE = bool(int(os.environ.get("DIMPA_TRACE", "0")))
LAST_TRACES = []


def _execute(nc, in_maps):
    r = run_bass_kernel_spmd(nc, in_maps, list(range(NCORES)), trace=TRACE)
    if TRACE:
        LAST_EXEC_NS.append(r.exec_time_ns)
        LAST_TRACES.append(r.instructions_and_trace)
    return r.results


def _round_up(a, b):
    return (a + b - 1) // b * b


def _block_col(a):
    nb = a.shape[0] // P
    return np.ascontiguousarray(
        a.reshape(nb, P, P).transpose(1, 0, 2).reshape(P, nb * P))


# ---------------------------------------------------------------- host prep

class EdgeLayout:
    """Per-direction packed edge metadata (no self loops).

    Slot order: per chunk of CH blocks -> [all blocks' lo slots][all hi
    slots] for the two gathers; meta arrays (w/dl/src) are block-major
    (per block: lo groups then hi groups)."""

    def __init__(self, row, col, ew, npad, bpc):
        half = npad // 2
        nblk = npad // P
        order = np.argsort(col, kind="stable")
        r = row[order].astype(np.int64)
        c_ = col[order].astype(np.int64)
        w = ew[order].astype(np.float32)
        blk = c_ // P
        starts = np.searchsorted(blk, np.arange(nblk + 1))
        lo_r, lo_w, lo_d = [], [], []
        hi_r, hi_w, hi_d = [], [], []
        cnt_lo = np.zeros(nblk, dtype=np.int64)
        cnt_hi = np.zeros(nblk, dtype=np.int64)
        for b in range(nblk):
            s, e = starts[b], starts[b + 1]
            rr, ww = r[s:e], w[s:e]
            dd = (c_[s:e] - b * P).astype(np.int64)
            m = rr < half
            lo_r.append(rr[m]); lo_w.append(ww[m]); lo_d.append(dd[m])
            hi_r.append(rr[~m] - half); hi_w.append(ww[~m]); hi_d.append(dd[~m])
            cnt_lo[b] = int(m.sum()); cnt_hi[b] = int((~m).sum())

        self.cap_lo = [max(_round_up(int(max(cnt_lo[c * bpc + jb]
                                             for c in range(NCORES))), P), P)
                       for jb in range(bpc)]
        self.cap_hi = [max(_round_up(int(max(cnt_hi[c * bpc + jb]
                                             for c in range(NCORES))), P), P)
                       for jb in range(bpc)]
        self.bpc = bpc
        self.half = half
        self.gw = sum(self.cap_lo[jb] + self.cap_hi[jb]
                      for jb in range(bpc)) // P
        self.iw = sum(self.cap_lo[jb] + self.cap_hi[jb]
                      for jb in range(bpc)) // 16
        # per-chunk gather sizes
        self.nch = bpc // CH
        self.NL = [sum(self.cap_lo[ch * CH + j] for j in range(CH))
                   for ch in range(self.nch)]
        self.NH = [sum(self.cap_hi[ch * CH + j] for j in range(CH))
                   for ch in range(self.nch)]

        self.idx, self.w, self.dl, self.srcg = [], [], [], []
        for c in range(NCORES):
            idx_p = np.zeros((P, self.iw), dtype=np.int16)
            w_p = np.zeros((P, self.gw), dtype=np.float32)
            dl_p = np.zeros((P, self.gw), dtype=np.int32)
            sg_p = np.zeros((P, self.gw), dtype=np.int64)
            io = go = 0
            for ch in range(self.nch):
                # gather-order: lo of all blocks, then hi of all blocks
                for (rows_l, cap_l, off) in ((lo_r, self.cap_lo, 0),
                                             (hi_r, self.cap_hi, half)):
                    v = []
                    for j in range(CH):
                        jb = ch * CH + j
                        b = c * bpc + jb
                        cap = cap_l[jb]
                        rr = np.zeros(cap, dtype=np.int16)
                        rr[:len(rows_l[b])] = rows_l[b]
                        v.append(rr)
                    v = np.concatenate(v)
                    idx_p[:, io:io + len(v) // 16] = np.tile(
                        v.reshape(len(v) // 16, 16).T, (8, 1))
                    io += len(v) // 16
                # meta order: per block, lo groups then hi groups
                for j in range(CH):
                    jb = ch * CH + j
                    b = c * bpc + jb
                    for (rows_l, ws_l, ds_l, cap, off) in (
                        (lo_r, lo_w, lo_d, self.cap_lo[jb], 0),
                        (hi_r, hi_w, hi_d, self.cap_hi[jb], half),
                    ):
                        n = len(rows_l[b])
                        ww = np.zeros(cap, dtype=np.float32)
                        dd = np.zeros(cap, dtype=np.int32)
                        ss = np.zeros(cap, dtype=np.int64)
                        ww[:n] = ws_l[b]
                        dd[:n] = ds_l[b].astype(np.int32)
                        ss[:n] = rows_l[b] + off
                        g = cap // P
                        w_p[:, go:go + g] = ww.reshape(g, P).T
                        dl_p[:, go:go + g] = dd.reshape(g, P).T
                        sg_p[:, go:go + g] = ss.reshape(g, P).T
                        go += g
            self.idx.append(idx_p); self.w.append(w_p)
            self.dl.append(dl_p); self.srcg.append(sg_p)


def _build_wbn(row, ew, npad, k):
    nblk = npad // P
    order = np.argsort(row, kind="stable")
    r = row[order].astype(np.int64)
    w = ew[order].astype(np.float32)
    starts = np.searchsorted(r, np.arange(npad + 1))
    cnt = starts[1:] - starts[:-1]
    assert cnt.max() <= k
    out = np.zeros((npad, k), dtype=np.float32)
    mask = np.arange(k)[None, :] < cnt[:, None]
    out[mask] = w
    return (out.reshape(nblk, P, k).transpose(1, 0, 2)
            .reshape(P, nblk * k).copy())


# ------------------------------------------------------------- device build

def _build_launchA(bpc, k):
    nc = bacc.Bacc(None)
    wbn = {d: nc.declare_dram_parameter(f"wbn_{d}", [P, bpc * k], F32,
                                        isOutput=False) for d in "st"}
    x_in = {d: nc.declare_dram_parameter(f"x_{d}", [P, bpc * P], F32,
                                         isOutput=False) for d in "st"}
    dinv_out = {d: nc.declare_dram_parameter(f"dinv_{d}", [P, bpc], F32,
                                             isOutput=True) for d in "st"}
    x16_out = {d: nc.declare_dram_parameter(f"x16_{d}", [bpc * P, P], F16,
                                            isOutput=True) for d in "st"}
    with tile.TileContext(nc) as tc:
        with (
            tc.tile_pool(name="a", bufs=2) as ap,
            tc.tile_pool(name="b", bufs=2) as bp,
        ):
            for d in "st":
                wt = ap.tile([P, bpc, k], F32, tag="wbn")
                nc.sync.dma_start(
                    out=wt[:], in_=wbn[d][:].rearrange("p (b k) -> p b k", k=k))
                deg = ap.tile([P, bpc], F32, tag="deg")
                nc.vector.tensor_reduce(out=deg[:], in_=wt[:],
                                        axis=mybir.AxisListType.X,
                                        op=mybir.AluOpType.add)
                nc.vector.tensor_scalar_add(out=deg[:], in0=deg[:],
                                            scalar1=FILL)
                dinv = ap.tile([P, bpc], F32, tag="dinv")
                nc.vector.reciprocal(out=dinv[:], in_=deg[:])
                nc.sync.dma_start(out=dinv_out[d][:], in_=dinv[:])

                xc = bp.tile([P, bpc, P], F32, tag="xc")
                nc.sync.dma_start(
                    out=xc[:], in_=x_in[d][:].rearrange("p (b f) -> p b f", f=P))
                x16 = bp.tile([P, bpc, P], F16, tag="x16")
                nc.vector.tensor_scalar_add(out=x16[:], in0=xc[:], scalar1=0.0)
                nc.sync.dma_start(
                    out=x16_out[d][:].rearrange("(b p) f -> p b f", p=P),
                    in_=x16[:])
    nc.finalize()
    return nc


def _emit_conv_launch(lay_s, lay_t, bpc, hop2, ws=None, wt=None):
    """Build hop launch. hop2=False: write c1 fp16. hop2=True: feat epilogue."""
    half = lay_s.half
    nc = bacc.Bacc(None, num_swdge_queues=4)
    lays = {"s": lay_s, "t": lay_t}
    tabs = {}
    eg = {}
    for d in "st":
        tabs[d] = (nc.declare_dram_parameter(f"tab_{d}_lo", [half, P], F16,
                                             isOutput=False),
                   nc.declare_dram_parameter(f"tab_{d}_hi", [half, P], F16,
                                             isOutput=False))
        L = lays[d]
        eg[f"idx_{d}"] = nc.declare_dram_parameter(f"idx_{d}", [P, L.iw], I16,
                                                   isOutput=False)
        eg[f"w_{d}"] = nc.declare_dram_parameter(f"w_{d}", [P, L.gw], F32,
                                                 isOutput=False)
        eg[f"dv_{d}"] = nc.declare_dram_parameter(f"dv_{d}", [P, L.gw], F32,
                                                  isOutput=False)
        eg[f"dl_{d}"] = nc.declare_dram_parameter(f"dl_{d}", [P, L.gw], I32,
                                                  isOutput=False)
        eg[f"xo_{d}"] = nc.declare_dram_parameter(f"xo_{d}", [P, bpc * P], F16,
                                                  isOutput=False)
        eg[f"dinv_{d}"] = nc.declare_dram_parameter(f"dinv_{d}", [P, bpc], F32,
                                                    isOutput=False)
    iota_in = nc.declare_dram_parameter("iota16", [P, P], I32, isOutput=False)
    ident_in = nc.declare_dram_parameter("ident16", [P, P], F16, isOutput=False)
    if hop2:
        xsl = {d: nc.declare_dram_parameter(f"xsl_{d}", [P, bpc * P], F32,
                                            isOutput=False) for d in "st"}
        out = nc.declare_dram_parameter("out", [bpc * P, 2 * P], F32,
                                        isOutput=True)
    else:
        c1o = {d: nc.declare_dram_parameter(f"c1_{d}", [bpc * P, P], F16,
                                            isOutput=True) for d in "st"}

    qn = [0]
    with tile.TileContext(nc) as tc:
        with (
            tc.tile_pool(name="const", bufs=1) as constp,
            tc.tile_pool(name="meta", bufs=2) as metap,
            tc.tile_pool(name="g", bufs=12) as gp,
            tc.tile_pool(name="m", bufs=3) as mp,
            tc.tile_pool(name="dg", bufs=3) as dgp,
            tc.tile_pool(name="xo", bufs=3) as xop,
            tc.tile_pool(name="epi", bufs=6) as epip,
            tc.tile_pool(name="ps", bufs=6, space="PSUM") as psp,
        ):
            iota_t = constp.tile([P, 1, P], I32)
            nc.sync.dma_start(out=iota_t[:, 0, :], in_=iota_in[:])
            ident_t = constp.tile([P, P], F16)
            nc.sync.dma_start(out=ident_t[:], in_=ident_in[:])

            for d, wcoef in (("s", ws), ("t", wt)):
                L = lays[d]
                dinv_t = constp.tile([P, bpc], F32, tag=f"dv{d}")
                nc.sync.dma_start(out=dinv_t[:], in_=eg[f"dinv_{d}"][:])
                wdself = constp.tile([P, bpc], F16, tag=f"wds{d}")
                nc.vector.tensor_scalar_mul(out=wdself[:], in0=dinv_t[:],
                                            scalar1=FILL)
                io = go = 0
                for ch in range(L.nch):
                    NL, NH = L.NL[ch], L.NH[ch]
                    W = sum((L.cap_lo[ch * CH + j] + L.cap_hi[ch * CH + j])
                            for j in range(CH)) // P
                    w_t = metap.tile([P, W], F32, tag="w")
                    nc.sync.dma_start(out=w_t[:],
                                      in_=eg[f"w_{d}"][:, go:go + W])
                    dv_t = metap.tile([P, W], F32, tag="dv")
                    nc.sync.dma_start(out=dv_t[:],
                                      in_=eg[f"dv_{d}"][:, go:go + W])
                    dl_t = metap.tile([P, W], I32, tag="dl")
                    nc.sync.dma_start(out=dl_t[:],
                                      in_=eg[f"dl_{d}"][:, go:go + W])
                    wd32 = metap.tile([P, W], F32, tag="wd")
                    nc.vector.tensor_tensor(out=wd32[:], in0=w_t[:],
                                            in1=dv_t[:],
                                            op=mybir.AluOpType.mult)

                    idx_l = metap.tile([P, NL // 16], I16, tag="il")
                    nc.sync.dma_start(out=idx_l[:],
                                      in_=eg[f"idx_{d}"][:, io:io + NL // 16])
                    io += NL // 16
                    idx_h = metap.tile([P, NH // 16], I16, tag="ih")
                    nc.sync.dma_start(out=idx_h[:],
                                      in_=eg[f"idx_{d}"][:, io:io + NH // 16])
                    io += NH // 16

                    # chunk-level prefetch of own-rows (diag rhs / epilogue)
                    xoc = xop.tile([P, CH, P], F16, tag="xoc")
                    nc.sync.dma_start(
                        out=xoc[:],
                        in_=eg[f"xo_{d}"][:, ch * CH * P:(ch + 1) * CH * P]
                        .rearrange("p (b f) -> p b f", f=P))
                    if hop2:
                        xtc = xop.tile([P, CH, P], F32, tag="xtc")
                        nc.sync.dma_start(
                            out=xtc[:],
                            in_=xsl[d][:, ch * CH * P:(ch + 1) * CH * P]
                            .rearrange("p (b f) -> p b f", f=P))

                    # paired gathers: 2 blocks per dma_gather call
                    xgl_t, xgh_t = [], []
                    il_off, ih_off = 0, 0
                    for j2 in range(0, CH, 2):
                        jb2 = ch * CH + j2
                        for (cap_l, tab, lst, off_ref) in (
                            (L.cap_lo, tabs[d][0], xgl_t, "l"),
                            (L.cap_hi, tabs[d][1], xgh_t, "h"),
                        ):
                            cc = cap_l[jb2] + cap_l[jb2 + 1]
                            t = gp.tile([P, cc // P, P], F16, tag="xg")
                            idx_t = idx_l if off_ref == "l" else idx_h
                            o = il_off if off_ref == "l" else ih_off
                            nc.gpsimd.dma_gather(
                                t[:], tab[:], idx_t[:, o:o + cc // 16],
                                cc, cc, P, single_packet=False,
                                queue_num=qn[0] % 4)
                            qn[0] += 1
                            lst.append(t)
                            if off_ref == "l":
                                il_off += cc // 16
                            else:
                                ih_off += cc // 16

                    boff = 0
                    for j in range(CH):
                        jb = ch * CH + j
                        g_lo = L.cap_lo[jb] // P
                        g_hi = L.cap_hi[jb] // P
                        g_tot = g_lo + g_hi
                        gl_base = (L.cap_lo[jb - 1] // P) if j % 2 else 0
                        gh_base = (L.cap_hi[jb - 1] // P) if j % 2 else 0
                        xgl = xgl_t[j // 2]
                        xgh = xgh_t[j // 2]
                        m32 = mp.tile([P, g_tot, P], F32, tag="m32")
                        nc.vector.tensor_tensor(
                            out=m32[:],
                            in0=iota_t[:].to_broadcast([P, g_tot, P]),
                            in1=dl_t[:, boff:boff + g_tot].to_broadcast(
                                [P, g_tot, P]),
                            op=mybir.AluOpType.is_equal)
                        m16 = mp.tile([P, g_tot, P], F16, tag="m16")
                        nc.vector.tensor_tensor(
                            out=m16[:], in0=m32[:],
                            in1=wd32[:, boff:boff + g_tot].to_broadcast(
                                [P, g_tot, P]),
                            op=mybir.AluOpType.mult)
                        diag = dgp.tile([P, P], F16, tag="diag")
                        nc.vector.tensor_tensor(
                            out=diag[:], in0=ident_t[:],
                            in1=wdself[:, jb:jb + 1].to_broadcast([P, P]),
                            op=mybir.AluOpType.mult)

                        ps = psp.tile([P, P], F32, space="PSUM", tag="ps")
                        for g in range(g_lo):
                            nc.tensor.matmul(out=ps[:], lhsT=m16[:, g, :],
                                             rhs=xgl[:, gl_base + g, :],
                                             start=(g == 0), stop=False)
                        for g in range(g_hi):
                            nc.tensor.matmul(out=ps[:], lhsT=m16[:, g_lo + g, :],
                                             rhs=xgh[:, gh_base + g, :],
                                             start=False, stop=False)
                        nc.tensor.matmul(out=ps[:], lhsT=diag[:],
                                         rhs=xoc[:, j, :],
                                         start=False, stop=True)

                        if not hop2:
                            c1t = epip.tile([P, P], F16, tag="c1")
                            nc.vector.tensor_scalar_add(out=c1t[:], in0=ps[:],
                                                        scalar1=0.0)
                            nc.sync.dma_start(
                                out=c1o[d][jb * P:(jb + 1) * P, :], in_=c1t[:])
                        else:
                            w0, w1, w2 = (float(wcoef[0]), float(wcoef[1]),
                                          float(wcoef[2]))
                            co = 0 if d == "s" else P
                            s0 = epip.tile([P, P], F32, tag="s0")
                            nc.vector.tensor_scalar_mul(out=s0[:],
                                                        in0=xtc[:, j, :],
                                                        scalar1=w0)
                            s1 = epip.tile([P, P], F32, tag="s1")
                            nc.vector.scalar_tensor_tensor(
                                out=s1[:], in0=xoc[:, j, :], scalar=w1,
                                in1=s0[:],
                                op0=mybir.AluOpType.mult,
                                op1=mybir.AluOpType.add)
                            ft = epip.tile([P, P], F32, tag="ft")
                            nc.vector.scalar_tensor_tensor(
                                out=ft[:], in0=ps[:], scalar=w2, in1=s1[:],
                                op0=mybir.AluOpType.mult,
                                op1=mybir.AluOpType.add)
                            nc.sync.dma_start(
                                out=out[jb * P:(jb + 1) * P, co:co + P],
                                in_=ft[:])
                        boff += g_tot
                    go += W
    nc.finalize()
    return nc


# ------------------------------------------------------------------ driver

def kernel(**inputs):
    x_s = np.ascontiguousarray(np.asarray(inputs["x_s"], dtype=np.float32))
    x_t = np.ascontiguousarray(np.asarray(inputs["x_t"], dtype=np.float32))
    edge_index = np.asarray(inputs["edge_index"])
    edge_weight = np.asarray(inputs["edge_weight"], dtype=np.float32)
    hop = 2
    ws = np.asarray(inputs.get("w_s", np.ones((hop + 1, 1))),
                    dtype=np.float32).ravel()
    wt = np.asarray(inputs.get("w_t", np.ones((hop + 1, 1))),
                    dtype=np.float32).ravel()

    n, dfeat = x_s.shape
    assert dfeat == P
    npad = _round_up(n, 2 * NCORES * P)
    half = npad // 2
    nblk = npad // P
    bpc = nblk // NCORES
    assert bpc % CH == 0
    src = edge_index[0].astype(np.int64)
    dst = edge_index[1].astype(np.int64)

    xs_p = np.zeros((npad, P), dtype=np.float32)
    xs_p[:n] = x_s
    xt_p = np.zeros((npad, P), dtype=np.float32)
    xt_p[:n] = x_t
    xs_bc = _block_col(xs_p)
    xt_bc = _block_col(xt_p)

    k = int(max(np.bincount(src, minlength=1).max(),
                np.bincount(dst, minlength=1).max()))
    k = _round_up(max(k, 4), 4)
    wbn_s = _build_wbn(src, edge_weight, npad, k)
    wbn_t = _build_wbn(dst, edge_weight, npad, k)

    lay_s = EdgeLayout(src, dst, edge_weight, npad, bpc)
    lay_t = EdgeLayout(dst, src, edge_weight, npad, bpc)

    iota16 = np.tile(np.arange(P, dtype=np.int32), (P, 1))
    ident16 = np.eye(P, dtype=np.float16)

    # ---- launch A: degrees + fp16 cast
    ncA = _build_launchA(bpc, k)
    in_mapsA = []
    for c in range(NCORES):
        in_mapsA.append({
            "wbn_s": np.ascontiguousarray(wbn_s[:, c * bpc * k:(c + 1) * bpc * k]),
            "wbn_t": np.ascontiguousarray(wbn_t[:, c * bpc * k:(c + 1) * bpc * k]),
            "x_s": np.ascontiguousarray(xs_bc[:, c * bpc * P:(c + 1) * bpc * P]),
            "x_t": np.ascontiguousarray(xt_bc[:, c * bpc * P:(c + 1) * bpc * P]),
        })
    resA = _execute(ncA, in_mapsA)

    dinv_full = {}
    tab16 = {}
    for d in "st":
        dinv_full[d] = np.concatenate(
            [resA[c][f"dinv_{d}"].T.reshape(-1) for c in range(NCORES)])
        tab16[d] = np.concatenate([resA[c][f"x16_{d}"] for c in range(NCORES)],
                                  axis=0)

    def dv_slot(lay, d):
        return [dinv_full[d][lay.srcg[c]].astype(np.float32)
                for c in range(NCORES)]

    dv_s = dv_slot(lay_s, "s")
    dv_t = dv_slot(lay_t, "t")

    def conv_maps(tabs, xo16):
        maps = []
        for c in range(NCORES):
            m = {"iota16": iota16, "ident16": ident16}
            for d, lay, dv in (("s", lay_s, dv_s), ("t", lay_t, dv_t)):
                m[f"tab_{d}_lo"] = np.ascontiguousarray(tabs[d][:half])
                m[f"tab_{d}_hi"] = np.ascontiguousarray(tabs[d][half:])
                m[f"idx_{d}"] = lay.idx[c]
                m[f"w_{d}"] = lay.w[c]
                m[f"dv_{d}"] = dv[c]
                m[f"dl_{d}"] = lay.dl[c]
                m[f"xo_{d}"] = _block_col(
                    xo16[d][c * bpc * P:(c + 1) * bpc * P])
                m[f"dinv_{d}"] = resA[c][f"dinv_{d}"]
            maps.append(m)
        return maps

    # ---- launch B: hop 1
    ncB = _emit_conv_launch(lay_s, lay_t, bpc, hop2=False)
    resB = _execute(ncB, conv_maps(tab16, tab16))

    c1 = {d: np.concatenate([resB[c][f"c1_{d}"] for c in range(NCORES)],
                            axis=0) for d in "st"}

    # ---- launch C: hop 2 + feat
    ncC = _emit_conv_launch(lay_s, lay_t, bpc, hop2=True, ws=ws, wt=wt)
    mapsC = conv_maps(c1, c1)
    for c in range(NCORES):
        mapsC[c]["xsl_s"] = np.ascontiguousarray(
            xs_bc[:, c * bpc * P:(c + 1) * bpc * P])
        mapsC[c]["xsl_t"] = np.ascontiguousarray(
            xt_bc[:, c * bpc * P:(c + 1) * bpc * P])
    resC = _execute(ncC, mapsC)

    out = np.concatenate([resC[c]["out"] for c in range(NCORES)], axis=0)
    return np.ascontiguousarray(out[:n]).astype(np.float32)


# revision 24
# speedup vs baseline: 1.0116x; 1.0116x over previous
"""DIMPA 2-hop directed message passing on 8 Trainium2 NeuronCores (Bass).

v2: deg-normalization folded into per-edge weights (w_norm = w * deg_inv[src]),
fp16 gather tables + fp16 one-hot matmuls, chunked dma_gather calls,
self-loops as a diagonal matmul (no gather slots).

Launches:
  A: per-core deg_inv (from weights grouped by node) + fp16 cast of own x rows.
  B: hop1 convs (gather x16, one-hot matmul accumulate, write c1 fp16).
  C: hop2 convs (gather c1 fp16) + feat epilogue -> out [N, 2D].
Host between launches only permutes / concatenates device-produced arrays.
"""

import os
import numpy as np
from concourse import bacc, mybir
import concourse.tile as tile
from concourse.bass_utils import run_bass_kernel_spmd

FILL = 0.5
NCORES = 8
P = 128
CH = 10                     # blocks per gather chunk
F32 = mybir.dt.float32
F16 = mybir.dt.float16
I16 = mybir.dt.int16
I32 = mybir.dt.int32

LAST_EXEC_NS = []
TRACE = bool(int(os.environ.get("DIMPA_TRACE", "0")))
LAST_TRACES = []


def _execute(nc, in_maps):
    r = run_bass_kernel_spmd(nc, in_maps, list(range(NCORES)), trace=TRACE)
    if TRACE:
        LAST_EXEC_NS.append(r.exec_time_ns)
        LAST_TRACES.append(r.instructions_and_trace)
    return r.results


def _round_up(a, b):
    return (a + b - 1) // b * b


def _block_col(a):
    nb = a.shape[0] // P
    return np.ascontiguousarray(
        a.reshape(nb, P, P).transpose(1, 0, 2).reshape(P, nb * P))


# ---------------------------------------------------------------- host prep

class EdgeLayout:
    """Per-direction packed edge metadata (no self loops).

    Slot order: per chunk of CH blocks -> [all blocks' lo slots][all hi
    slots] for the two gathers; meta arrays (w/dl/src) are block-major
    (per block: lo groups then hi groups)."""

    def __init__(self, row, col, ew, npad, bpc):
        half = npad // 2
        nblk = npad // P
        order = np.argsort(col, kind="stable")
        r = row[order].astype(np.int64)
        c_ = col[order].astype(np.int64)
        w = ew[order].astype(np.float32)
        blk = c_ // P
        starts = np.searchsorted(blk, np.arange(nblk + 1))
        lo_r, lo_w, lo_d = [], [], []
        hi_r, hi_w, hi_d = [], [], []
        cnt_lo = np.zeros(nblk, dtype=np.int64)
        cnt_hi = np.zeros(nblk, dtype=np.int64)
        for b in range(nblk):
            s, e = starts[b], starts[b + 1]
            rr, ww = r[s:e], w[s:e]
            dd = (c_[s:e] - b * P).astype(np.int64)
            m = rr < half
            lo_r.append(rr[m]); lo_w.append(ww[m]); lo_d.append(dd[m])
            hi_r.append(rr[~m] - half); hi_w.append(ww[~m]); hi_d.append(dd[~m])
            cnt_lo[b] = int(m.sum()); cnt_hi[b] = int((~m).sum())

        self.cap_lo = [max(_round_up(int(max(cnt_lo[c * bpc + jb]
                                             for c in range(NCORES))), P), P)
                       for jb in range(bpc)]
        self.cap_hi = [max(_round_up(int(max(cnt_hi[c * bpc + jb]
                                             for c in range(NCORES))), P), P)
                       for jb in range(bpc)]
        self.bpc = bpc
        self.half = half
        self.gw = sum(self.cap_lo[jb] + self.cap_hi[jb]
                      for jb in range(bpc)) // P
        self.iw = sum(self.cap_lo[jb] + self.cap_hi[jb]
                      for jb in range(bpc)) // 16
        # per-chunk gather sizes
        self.nch = bpc // CH
        self.NL = [sum(self.cap_lo[ch * CH + j] for j in range(CH))
                   for ch in range(self.nch)]
        self.NH = [sum(self.cap_hi[ch * CH + j] for j in range(CH))
                   for ch in range(self.nch)]

        self.idx, self.w, self.dl, self.srcg = [], [], [], []
        for c in range(NCORES):
            idx_p = np.zeros((P, self.iw), dtype=np.int16)
            w_p = np.zeros((P, self.gw), dtype=np.float32)
            dl_p = np.zeros((P, self.gw), dtype=np.int32)
            sg_p = np.zeros((P, self.gw), dtype=np.int64)
            io = go = 0
            for ch in range(self.nch):
                # gather-order: lo of all blocks, then hi of all blocks
                for (rows_l, cap_l, off) in ((lo_r, self.cap_lo, 0),
                                             (hi_r, self.cap_hi, half)):
                    v = []
                    for j in range(CH):
                        jb = ch * CH + j
                        b = c * bpc + jb
                        cap = cap_l[jb]
                        rr = np.zeros(cap, dtype=np.int16)
                        rr[:len(rows_l[b])] = rows_l[b]
                        v.append(rr)
                    v = np.concatenate(v)
                    idx_p[:, io:io + len(v) // 16] = np.tile(
                        v.reshape(len(v) // 16, 16).T, (8, 1))
                    io += len(v) // 16
                # meta order: per block, lo groups then hi groups
                for j in range(CH):
                    jb = ch * CH + j
                    b = c * bpc + jb
                    for (rows_l, ws_l, ds_l, cap, off) in (
                        (lo_r, lo_w, lo_d, self.cap_lo[jb], 0),
                        (hi_r, hi_w, hi_d, self.cap_hi[jb], half),
                    ):
                        n = len(rows_l[b])
                        ww = np.zeros(cap, dtype=np.float32)
                        dd = np.zeros(cap, dtype=np.int32)
                        ss = np.zeros(cap, dtype=np.int64)
                        ww[:n] = ws_l[b]
                        dd[:n] = ds_l[b].astype(np.int32)
                        ss[:n] = rows_l[b] + off
                        g = cap // P
                        w_p[:, go:go + g] = ww.reshape(g, P).T
                        dl_p[:, go:go + g] = dd.reshape(g, P).T
                        sg_p[:, go:go + g] = ss.reshape(g, P).T
                        go += g
            self.idx.append(idx_p); self.w.append(w_p)
            self.dl.append(dl_p); self.srcg.append(sg_p)


def _build_wbn(row, ew, npad, k):
    nblk = npad // P
    order = np.argsort(row, kind="stable")
    r = row[order].astype(np.int64)
    w = ew[order].astype(np.float32)
    starts = np.searchsorted(r, np.arange(npad + 1))
    cnt = starts[1:] - starts[:-1]
    assert cnt.max() <= k
    out = np.zeros((npad, k), dtype=np.float32)
    mask = np.arange(k)[None, :] < cnt[:, None]
    out[mask] = w
    return (out.reshape(nblk, P, k).transpose(1, 0, 2)
            .reshape(P, nblk * k).copy())


# ------------------------------------------------------------- device build

def _build_launchA(bpc, k):
    nc = bacc.Bacc(None)
    wbn = {d: nc.declare_dram_parameter(f"wbn_{d}", [P, bpc * k], F32,
                                        isOutput=False) for d in "st"}
    x_in = {d: nc.declare_dram_parameter(f"x_{d}", [P, bpc * P], F32,
                                         isOutput=False) for d in "st"}
    dinv_out = {d: nc.declare_dram_parameter(f"dinv_{d}", [P, bpc], F32,
                                             isOutput=True) for d in "st"}
    x16_out = {d: nc.declare_dram_parameter(f"x16_{d}", [bpc * P, P], F16,
                                            isOutput=True) for d in "st"}
    with tile.TileContext(nc) as tc:
        with (
            tc.tile_pool(name="a", bufs=2) as ap,
            tc.tile_pool(name="b", bufs=2) as bp,
        ):
            for d in "st":
                wt = ap.tile([P, bpc, k], F32, tag="wbn")
                nc.sync.dma_start(
                    out=wt[:], in_=wbn[d][:].rearrange("p (b k) -> p b k", k=k))
                deg = ap.tile([P, bpc], F32, tag="deg")
                nc.vector.tensor_reduce(out=deg[:], in_=wt[:],
                                        axis=mybir.AxisListType.X,
                                        op=mybir.AluOpType.add)
                nc.vector.tensor_scalar_add(out=deg[:], in0=deg[:],
                                            scalar1=FILL)
                dinv = ap.tile([P, bpc], F32, tag="dinv")
                nc.vector.reciprocal(out=dinv[:], in_=deg[:])
                nc.sync.dma_start(out=dinv_out[d][:], in_=dinv[:])

                xc = bp.tile([P, bpc, P], F32, tag="xc")
                nc.sync.dma_start(
                    out=xc[:], in_=x_in[d][:].rearrange("p (b f) -> p b f", f=P))
                x16 = bp.tile([P, bpc, P], F16, tag="x16")
                nc.vector.tensor_scalar_add(out=x16[:], in0=xc[:], scalar1=0.0)
                nc.sync.dma_start(
                    out=x16_out[d][:].rearrange("(b p) f -> p b f", p=P),
                    in_=x16[:])
    nc.finalize()
    return nc


def _emit_conv_launch(lay_s, lay_t, bpc, hop2, ws=None, wt=None):
    """Build hop launch. hop2=False: write c1 fp16. hop2=True: feat epilogue."""
    half = lay_s.half
    nc = bacc.Bacc(None, num_swdge_queues=4)
    lays = {"s": lay_s, "t": lay_t}
    tabs = {}
    eg = {}
    for d in "st":
        tabs[d] = (nc.declare_dram_parameter(f"tab_{d}_lo", [half, P], F16,
                                             isOutput=False),
                   nc.declare_dram_parameter(f"tab_{d}_hi", [half, P], F16,
                                             isOutput=False))
        L = lays[d]
        eg[f"idx_{d}"] = nc.declare_dram_parameter(f"idx_{d}", [P, L.iw], I16,
                                                   isOutput=False)
        eg[f"w_{d}"] = nc.declare_dram_parameter(f"w_{d}", [P, L.gw], F32,
                                                 isOutput=False)
        eg[f"dv_{d}"] = nc.declare_dram_parameter(f"dv_{d}", [P, L.gw], F32,
                                                  isOutput=False)
        eg[f"dl_{d}"] = nc.declare_dram_parameter(f"dl_{d}", [P, L.gw], I32,
                                                  isOutput=False)
        eg[f"xo_{d}"] = nc.declare_dram_parameter(f"xo_{d}", [P, bpc * P], F16,
                                                  isOutput=False)
        eg[f"dinv_{d}"] = nc.declare_dram_parameter(f"dinv_{d}", [P, bpc], F32,
                                                    isOutput=False)
    iota_in = nc.declare_dram_parameter("iota16", [P, P], I32, isOutput=False)
    ident_in = nc.declare_dram_parameter("ident16", [P, P], F16, isOutput=False)
    if hop2:
        xsl = {d: nc.declare_dram_parameter(f"xsl_{d}", [P, bpc * P], F32,
                                            isOutput=False) for d in "st"}
        out = nc.declare_dram_parameter("out", [bpc * P, 2 * P], F32,
                                        isOutput=True)
    else:
        c1o = {d: nc.declare_dram_parameter(f"c1_{d}", [bpc * P, P], F16,
                                            isOutput=True) for d in "st"}

    qn = [0]
    with tile.TileContext(nc) as tc:
        with (
            tc.tile_pool(name="const", bufs=1) as constp,
            tc.tile_pool(name="meta", bufs=4) as metap,
            tc.tile_pool(name="g", bufs=12) as gp,
            tc.tile_pool(name="m", bufs=3) as mp,
            tc.tile_pool(name="dg", bufs=3) as dgp,
            tc.tile_pool(name="xo", bufs=4) as xop,
            tc.tile_pool(name="epi", bufs=6) as epip,
            tc.tile_pool(name="ps", bufs=6, space="PSUM") as psp,
        ):
            iota_t = constp.tile([P, 1, P], I32)
            nc.sync.dma_start(out=iota_t[:, 0, :], in_=iota_in[:])
            ident_t = constp.tile([P, P], F16)
            nc.sync.dma_start(out=ident_t[:], in_=ident_in[:])

            for d, wcoef in (("s", ws), ("t", wt)):
                L = lays[d]
                dinv_t = constp.tile([P, bpc], F32, tag=f"dv{d}")
                nc.sync.dma_start(out=dinv_t[:], in_=eg[f"dinv_{d}"][:])
                wdself = constp.tile([P, bpc], F16, tag=f"wds{d}")
                nc.vector.tensor_scalar_mul(out=wdself[:], in0=dinv_t[:],
                                            scalar1=FILL)
                io = go = 0
                for ch in range(L.nch):
                    NL, NH = L.NL[ch], L.NH[ch]
                    W = sum((L.cap_lo[ch * CH + j] + L.cap_hi[ch * CH + j])
                            for j in range(CH)) // P
                    idx_l = metap.tile([P, NL // 16], I16, tag="il")
                    nc.sync.dma_start(out=idx_l[:],
                                      in_=eg[f"idx_{d}"][:, io:io + NL // 16])
                    io += NL // 16
                    idx_h = metap.tile([P, NH // 16], I16, tag="ih")
                    nc.sync.dma_start(out=idx_h[:],
                                      in_=eg[f"idx_{d}"][:, io:io + NH // 16])
                    io += NH // 16

                    # chunk-level prefetch of own-rows (diag rhs / epilogue),
                    # issued on the otherwise-idle Scalar engine
                    xoc = xop.tile([P, CH, P], F16, tag="xoc")
                    nc.scalar.dma_start(
                        out=xoc[:],
                        in_=eg[f"xo_{d}"][:, ch * CH * P:(ch + 1) * CH * P]
                        .rearrange("p (b f) -> p b f", f=P))
                    if hop2:
                        xtc = xop.tile([P, CH, P], F32, tag="xtc")
                        nc.scalar.dma_start(
                            out=xtc[:],
                            in_=xsl[d][:, ch * CH * P:(ch + 1) * CH * P]
                            .rearrange("p (b f) -> p b f", f=P))

                    # paired gathers: 2 blocks per dma_gather call
                    xgl_t, xgh_t = [], []
                    il_off, ih_off = 0, 0
                    for j2 in range(0, CH, 2):
                        jb2 = ch * CH + j2
                        for (cap_l, tab, lst, off_ref) in (
                            (L.cap_lo, tabs[d][0], xgl_t, "l"),
                            (L.cap_hi, tabs[d][1], xgh_t, "h"),
                        ):
                            cc = cap_l[jb2] + cap_l[jb2 + 1]
                            t = gp.tile([P, cc // P, P], F16, tag="xg")
                            idx_t = idx_l if off_ref == "l" else idx_h
                            o = il_off if off_ref == "l" else ih_off
                            nc.gpsimd.dma_gather(
                                t[:], tab[:], idx_t[:, o:o + cc // 16],
                                cc, cc, P, single_packet=False,
                                queue_num=qn[0] % 4)
                            qn[0] += 1
                            lst.append(t)
                            if off_ref == "l":
                                il_off += cc // 16
                            else:
                                ih_off += cc // 16

                    boff = 0
                    for j in range(CH):
                        jb = ch * CH + j
                        g_lo = L.cap_lo[jb] // P
                        g_hi = L.cap_hi[jb] // P
                        g_tot = g_lo + g_hi
                        gl_base = (L.cap_lo[jb - 1] // P) if j % 2 else 0
                        gh_base = (L.cap_hi[jb - 1] // P) if j % 2 else 0
                        xgl = xgl_t[j // 2]
                        xgh = xgh_t[j // 2]
                        w_t = metap.tile([P, g_tot], F32, tag="w")
                        nc.scalar.dma_start(
                            out=w_t[:],
                            in_=eg[f"w_{d}"][:, go + boff:go + boff + g_tot])
                        dv_t = metap.tile([P, g_tot], F32, tag="dv")
                        nc.scalar.dma_start(
                            out=dv_t[:],
                            in_=eg[f"dv_{d}"][:, go + boff:go + boff + g_tot])
                        dl_t = metap.tile([P, g_tot], I32, tag="dl")
                        nc.scalar.dma_start(
                            out=dl_t[:],
                            in_=eg[f"dl_{d}"][:, go + boff:go + boff + g_tot])
                        wd32 = metap.tile([P, g_tot], F32, tag="wd")
                        nc.vector.tensor_tensor(out=wd32[:], in0=w_t[:],
                                                in1=dv_t[:],
                                                op=mybir.AluOpType.mult)
                        m32 = mp.tile([P, g_tot, P], F32, tag="m32")
                        nc.vector.tensor_tensor(
                            out=m32[:],
                            in0=iota_t[:].to_broadcast([P, g_tot, P]),
                            in1=dl_t[:].to_broadcast([P, g_tot, P]),
                            op=mybir.AluOpType.is_equal)
                        m16 = mp.tile([P, g_tot, P], F16, tag="m16")
                        nc.vector.tensor_tensor(
                            out=m16[:], in0=m32[:],
                            in1=wd32[:].to_broadcast([P, g_tot, P]),
                            op=mybir.AluOpType.mult)
                        diag = dgp.tile([P, P], F16, tag="diag")
                        nc.vector.tensor_tensor(
                            out=diag[:], in0=ident_t[:],
                            in1=wdself[:, jb:jb + 1].to_broadcast([P, P]),
                            op=mybir.AluOpType.mult)

                        ps = psp.tile([P, P], F32, space="PSUM", tag="ps")
                        for g in range(g_lo):
                            nc.tensor.matmul(out=ps[:], lhsT=m16[:, g, :],
                                             rhs=xgl[:, gl_base + g, :],
                                             start=(g == 0), stop=False)
                        for g in range(g_hi):
                            nc.tensor.matmul(out=ps[:], lhsT=m16[:, g_lo + g, :],
                                             rhs=xgh[:, gh_base + g, :],
                                             start=False, stop=False)
                        nc.tensor.matmul(out=ps[:], lhsT=diag[:],
                                         rhs=xoc[:, j, :],
                                         start=False, stop=True)

                        if not hop2:
                            c1t = epip.tile([P, P], F16, tag="c1")
                            nc.vector.tensor_scalar_add(out=c1t[:], in0=ps[:],
                                                        scalar1=0.0)
                            nc.sync.dma_start(
                                out=c1o[d][jb * P:(jb + 1) * P, :], in_=c1t[:])
                        else:
                            w0, w1, w2 = (float(wcoef[0]), float(wcoef[1]),
                                          float(wcoef[2]))
                            co = 0 if d == "s" else P
                            s0 = epip.tile([P, P], F32, tag="s0")
                            nc.vector.tensor_scalar_mul(out=s0[:],
                                                        in0=xtc[:, j, :],
                                                        scalar1=w0)
                            s1 = epip.tile([P, P], F32, tag="s1")
                            nc.vector.scalar_tensor_tensor(
                                out=s1[:], in0=xoc[:, j, :], scalar=w1,
                                in1=s0[:],
                                op0=mybir.AluOpType.mult,
                                op1=mybir.AluOpType.add)
                            ft = epip.tile([P, P], F32, tag="ft")
                            nc.vector.scalar_tensor_tensor(
                                out=ft[:], in0=ps[:], scalar=w2, in1=s1[:],
                                op0=mybir.AluOpType.mult,
                                op1=mybir.AluOpType.add)
                            nc.sync.dma_start(
                                out=out[jb * P:(jb + 1) * P, co:co + P],
                                in_=ft[:])
                        boff += g_tot
                    go += W
    nc.finalize()
    return nc


# ------------------------------------------------------------------ driver

def kernel(**inputs):
    x_s = np.ascontiguousarray(np.asarray(inputs["x_s"], dtype=np.float32))
    x_t = np.ascontiguousarray(np.asarray(inputs["x_t"], dtype=np.float32))
    edge_index = np.asarray(inputs["edge_index"])
    edge_weight = np.asarray(inputs["edge_weight"], dtype=np.float32)
    hop = 2
    ws = np.asarray(inputs.get("w_s", np.ones((hop + 1, 1))),
                    dtype=np.float32).ravel()
    wt = np.asarray(inputs.get("w_t", np.ones((hop + 1, 1))),
                    dtype=np.float32).ravel()

    n, dfeat = x_s.shape
    assert dfeat == P
    npad = _round_up(n, 2 * NCORES * P)
    half = npad // 2
    nblk = npad // P
    bpc = nblk // NCORES
    assert bpc % CH == 0
    src = edge_index[0].astype(np.int64)
    dst = edge_index[1].astype(np.int64)

    xs_p = np.zeros((npad, P), dtype=np.float32)
    xs_p[:n] = x_s
    xt_p = np.zeros((npad, P), dtype=np.float32)
    xt_p[:n] = x_t
    xs_bc = _block_col(xs_p)
    xt_bc = _block_col(xt_p)

    k = int(max(np.bincount(src, minlength=1).max(),
                np.bincount(dst, minlength=1).max()))
    k = _round_up(max(k, 4), 4)
    wbn_s = _build_wbn(src, edge_weight, npad, k)
    wbn_t = _build_wbn(dst, edge_weight, npad, k)

    lay_s = EdgeLayout(src, dst, edge_weight, npad, bpc)
    lay_t = EdgeLayout(dst, src, edge_weight, npad, bpc)

    iota16 = np.tile(np.arange(P, dtype=np.int32), (P, 1))
    ident16 = np.eye(P, dtype=np.float16)

    # ---- launch A: degrees + fp16 cast
    ncA = _build_launchA(bpc, k)
    in_mapsA = []
    for c in range(NCORES):
        in_mapsA.append({
            "wbn_s": np.ascontiguousarray(wbn_s[:, c * bpc * k:(c + 1) * bpc * k]),
            "wbn_t": np.ascontiguousarray(wbn_t[:, c * bpc * k:(c + 1) * bpc * k]),
            "x_s": np.ascontiguousarray(xs_bc[:, c * bpc * P:(c + 1) * bpc * P]),
            "x_t": np.ascontiguousarray(xt_bc[:, c * bpc * P:(c + 1) * bpc * P]),
        })
    resA = _execute(ncA, in_mapsA)

    dinv_full = {}
    tab16 = {}
    for d in "st":
        dinv_full[d] = np.concatenate(
            [resA[c][f"dinv_{d}"].T.reshape(-1) for c in range(NCORES)])
        tab16[d] = np.concatenate([resA[c][f"x16_{d}"] for c in range(NCORES)],
                                  axis=0)

    def dv_slot(lay, d):
        return [dinv_full[d][lay.srcg[c]].astype(np.float32)
                for c in range(NCORES)]

    dv_s = dv_slot(lay_s, "s")
    dv_t = dv_slot(lay_t, "t")

    def conv_maps(tabs, xo16):
        maps = []
        for c in range(NCORES):
            m = {"iota16": iota16, "ident16": ident16}
            for d, lay, dv in (("s", lay_s, dv_s), ("t", lay_t, dv_t)):
                m[f"tab_{d}_lo"] = np.ascontiguousarray(tabs[d][:half])
                m[f"tab_{d}_hi"] = np.ascontiguousarray(tabs[d][half:])
                m[f"idx_{d}"] = lay.idx[c]
                m[f"w_{d}"] = lay.w[c]
                m[f"dv_{d}"] = dv[c]
                m[f"dl_{d}"] = lay.dl[c]
                m[f"xo_{d}"] = _block_col(
                    xo16[d][c * bpc * P:(c + 1) * bpc * P])
                m[f"dinv_{d}"] = resA[c][f"dinv_{d}"]
            maps.append(m)
        return maps

    # ---- launch B: hop 1
    ncB = _emit_conv_launch(lay_s, lay_t, bpc, hop2=False)
    resB = _execute(ncB, conv_maps(tab16, tab16))

    c1 = {d: np.concatenate([resB[c][f"c1_{d}"] for c in range(NCORES)],
                            axis=0) for d in "st"}

    # ---- launch C: hop 2 + feat
    ncC = _emit_conv_launch(lay_s, lay_t, bpc, hop2=True, ws=ws, wt=wt)
    mapsC = conv_maps(c1, c1)
    for c in range(NCORES):
        mapsC[c]["xsl_s"] = np.ascontiguousarray(
            xs_bc[:, c * bpc * P:(c + 1) * bpc * P])
        mapsC[c]["xsl_t"] = np.ascontiguousarray(
            xt_bc[:, c * bpc * P:(c + 1) * bpc * P])
    resC = _execute(ncC, mapsC)

    out = np.concatenate([resC[c]["out"] for c in range(NCORES)], axis=0)
    return np.ascontiguousarray(out[:n]).astype(np.float32)
